# revision 25
# baseline (speedup 1.0000x reference)
"""Entmax-bisect (alpha-entmax via 10-step bisection) on Trainium2.

Data-parallel over 8 NeuronCores: X [8, 2048, 4096] is sharded on the
leading dim (2048 rows x 4096 per core); the reduction dim stays local.
alpha is a replicated scalar folded into compile-time constants.

Math (per row, alpha=1.5 => am1=0.5):
    Xs = am1*X; mx = max(Xs); tau_lo = mx-1; tau_hi = mx-(1/d)^am1
    f(t) = sum(relu(Xs-t)^2) - 1;  10 bisection steps; out = p/sum(p)

On-device we work in the tau-hat domain (tau/am1), bit-exact w.r.t. the
reference when am1 is a power of two. Three-engine pipeline, G row-tiles
marching in lockstep through the bisection so every engine always has
independent work:
  class D pass: GPSIMD r=max(x,th)-th  ->  ACT p=(am1*r)^2 (+row-sum)
  class B pass: one fused custom DVE op p=relu((x-th)*am1)^2 (+row-sum)
  per-row scalar updates: fused custom DVE ops on [128,1] tiles
"""

import math
from operator import add as _op_add

import numpy as np

import concourse.bass as bass  # noqa: F401
import concourse.tile as tile
from concourse import bacc, mybir
from concourse.bass_utils import run_bass_kernel_spmd

N_CORES = 8
D = 4096
N_ITER = 10
P = 128

G = 5          # tiles marching together
B_MOD = 16     # class-B rotation modulus
B_THR = 1      # passes per B_MOD that run class B (fused on DVE)
NORM_ACT_MOD = 0   # >0: tiles with t % mod == 0 normalize on ACT, not DVE

TRACE = False
LAST_RESULT = None

_NC_CACHE = {}


# ---------- runtime registration of custom DVE ops ----------------------

def _register_dve_op(op_name, spec):
    from concourse import dve_ops as DO
    from concourse.dve_spec import lower, _has_src1 as has_src1
    from concourse.dve_uop import DveOpSpec

    for o in DO.OPS:
        if o.name == op_name:
            return o
    row = DO._CUSTOM_DVE_ROW_BASE + len(DO.OPS)
    assert row < 0x20
    shas = {}
    for ver in ("v3", "v4"):
        s = DveOpSpec(name=op_name, opcode=row, uops=lower(spec, ver=ver),
                      rd1_en=has_src1(spec))
        shas[ver] = s.sha(ver)
    op = DO.DveOp(op_name, spec, subdim=False, uops_sha=shas)
    DO.OPS.append(op)
    DO._SUB_OPCODE_FOR_NAME[op_name] = row
    DO.CUSTOM_DVE_SPECS[op_name] = spec
    return op


def _get_ops():
    from concourse.dve_spec import (
        Spec, Src0, Src1, C0, C1, C2, Zero, relu, select, sq,
    )

    def _ref_step(in0, in1, c0, c1, c2):
        b = np.maximum((in0.astype(np.float32) - c0) * c2, 0.0) ** 2
        b = b.astype(np.float32)
        return b, c1 + b.reshape(b.shape[0], -1).sum(axis=-1, keepdims=True)

    # out = relu((x - th)*am1)^2 ; accum = init + sum(out)
    step = _register_dve_op(
        "ENTMAX_STEP_ANT",
        Spec(body=sq(relu((Src0 - C0) * C2)), accum=_op_add, accum_init=C1,
             reference=_ref_step),
    )
    # out = select((f + imm2)*flo >= 0, tm, tlo)
    upd = _register_dve_op(
        "ENTMAX_TAU_UPD_ANT",
        Spec(body=select((Src0 + C2) * C0 >= Zero, Src1, C1),
             reference=lambda in0, in1, s0, s1, imm2: np.where(
                 (in0 + imm2) * s0 >= 0, in1, s1).astype(np.float32)),
    )
    # out = (mx - c_hi) - tlo        (per-row dm0, matching fp32 ref order)
    sub2 = _register_dve_op(
        "ENTMAX_SUB2_ANT",
        Spec(body=(Src0 - C0) - Src1,
             reference=lambda in0, in1, s0, s1, imm2: (
                 (in0.astype(np.float32) - s0) - in1).astype(np.float32)),
    )
    from concourse.dve_ops import AFFINE_THEN_ADD
    return step, upd, sub2, AFFINE_THEN_ADD


def _is_b(t_global, k):
    # class-B (fused DVE) passes; the rest are GPSIMD->ACT
    return ((k + 3 * t_global) % B_MOD) < B_THR


def _build(am1: float, rows: int):
    """Build the single-core Bass program for a [rows, D] shard."""
    f32 = mybir.dt.float32
    AF = mybir.ActivationFunctionType
    OP = mybir.AluOpType
    AX = mybir.AxisListType
    STEP, UPD, SUB2, AFF = _get_ops()

    # tau-hat domain constants (exact when am1 is a power of two)
    c_lo = 1.0 / am1
    pw = float(np.power(np.float32(1.0 / D), np.float32(am1)))
    c_hi = pw / am1

    nc = bacc.Bacc(None, target_bir_lowering=False)
    Xd = nc.declare_dram_parameter("X", [rows, D], f32, isOutput=False)
    Od = nc.declare_dram_parameter("OUT", [rows, D], f32, isOutput=True)
    ntiles = rows // P

    with tile.TileContext(nc) as tc:
        with (
            tc.tile_pool(name="xp", bufs=6) as xp,
            tc.tile_pool(name="rp", bufs=6) as rp,
            tc.tile_pool(name="st", bufs=8) as st,
        ):
            xt, rt, mx, tlo, dm0, ssum = {}, {}, {}, {}, {}, {}

            def emit_dma(t):
                xt[t] = xp.tile([P, D], f32, tag="xt", name="xt")
                nc.sync.dma_start(out=xt[t][:], in_=Xd[t * P:(t + 1) * P, :])

            def emit_max(t):
                mx[t] = st.tile([P, 1], f32, tag="mx", name="mx")
                nc.vector.reduce_max(mx[t][:], xt[t][:], axis=AX.X)

            def emit_setup(t):
                tlo[t] = st.tile([P, 1], f32, tag="tlo", name="tlo")
                nc.vector.tensor_scalar(tlo[t][:], mx[t][:], c_lo, None,
                                        OP.subtract)
                dm0[t] = st.tile([P, 1], f32, tag="dm0", name="dm0")
                nc.vector._custom_dve(SUB2, out=dm0[t][:], in0=mx[t][:],
                                      in1=tlo[t][:], s0=c_hi, s1=0.0,
                                      imm2=0.0)
                rt[t] = rp.tile([P, D], f32, tag="rt", name="rt")

            def emit_pass(t, k, force_b=False):
                # tau_m = tlo + dm0 * 2^-k   (dm halving is exact)
                tau = st.tile([P, 1], f32, tag="tm", name="tm")
                nc.vector._custom_dve(AFF, out=tau[:], in0=dm0[t][:],
                                      in1=tlo[t][:], s0=0.5 ** k, s1=0.0,
                                      imm2=0.0)
                last = k == N_ITER
                # acc: f = sum-1 for bisection steps (class B), raw sum
                # for the final pass / class A (f_bias compensates)
                acc = st.tile([P, 1], f32, tag="acc", name="acc")
                if force_b or _is_b(t, k):
                    nc.vector._custom_dve(
                        STEP, out=rt[t][:], in0=xt[t][:], s0=tau[:],
                        s1=(0.0 if last else -1.0), imm2=am1,
                        accum_out=acc[:])
                    f_bias = 0.0
                else:
                    nc.vector.tensor_scalar(rt[t][:], xt[t][:], tau[:],
                                            tau[:], OP.max, OP.subtract)
                    nc.scalar.activation(rt[t][:], rt[t][:], AF.Square,
                                         bias=0.0, scale=am1,
                                         accum_out=acc[:])
                    f_bias = -1.0
                if not last:
                    # mask = f_m >= 0  (f_lo >= 0 always; f_lo==0 rows give
                    # identical one-hot output either way, so the f_lo pass
                    # is dropped and its factor replaced by 1.0)
                    tlo_new = st.tile([P, 1], f32, tag="tlo", name="tlo")
                    nc.vector._custom_dve(
                        UPD, out=tlo_new[:], in0=acc[:], in1=tau[:],
                        s0=1.0, s1=tlo[t][:], imm2=f_bias)
                    tlo[t] = tlo_new
                else:
                    ssum[t] = acc

            def emit_teardown(t):
                rr = st.tile([P, 1], f32, tag="rr", name="rr")
                nc.vector.reciprocal(rr[:], ssum[t][:])
                if NORM_ACT_MOD > 0 and t % NORM_ACT_MOD == 0:
                    nc.scalar.mul(rt[t][:], rt[t][:], rr[:])
                else:
                    nc.vector.tensor_scalar(rt[t][:], rt[t][:], rr[:],
                                            None, OP.mult)
                nc.gpsimd.dma_start(out=Od[t * P:(t + 1) * P, :],
                                    in_=rt[t][:])

            # Skewed software pipeline: tile t runs pass k at step
            # s = off[t] + k, so ~N_ITER/stride tiles are mid-flight at any
            # step and setup/teardown work is spread evenly. Stride 1 at
            # both ends compresses the pipeline ramp/drain.
            offs = [2 * t for t in range(ntiles)]
            for s in range(-1, (offs[-1] if offs else 0) + N_ITER + 2):
                for t in range(ntiles):
                    k = s - offs[t]
                    if k == -1:
                        emit_dma(t)
                    elif k == 0:
                        emit_max(t)
                        emit_setup(t)
                    elif 1 <= k <= N_ITER:
                        # ramp/drain edges run solo: the fused DVE pass has a
                        # shorter serial chain than relu->ACT, so force class
                        # B where few tiles are in flight
                        solo = (t == 0 and k <= 2) or (t == 1 and k <= 1) or \
                               (t == ntiles - 1 and k >= 6) or \
                               (t == ntiles - 2 and k >= 9)
                        emit_pass(t, k, force_b=solo)
                    elif k == N_ITER + 1:
                        emit_teardown(t)

    nc.finalize()
    return nc


def _get_nc(am1: float, rows: int):
    key = (am1, rows, G, B_MOD, B_THR, NORM_ACT_MOD)
    if key not in _NC_CACHE:
        _NC_CACHE[key] = _build(am1, rows)
    return _NC_CACHE[key]


def _ensure_ntff_hook():
    """Register the NTFF profile hook that bass_utils needs for trace=True
    under axon (this image's antenv lacks axon_hooks; build it from the
    boot shim's ctypes driver). Also neuter the S3 artifact upload."""
    import sys as _sys
    import types

    import antenv
    import concourse.bass_utils as _bu

    _bu.upload_artifacts = lambda tmpdir: str(tmpdir)
    try:
        from antenv import axon_hooks  # noqa: F401
        return
    except ImportError:
        pass
    from trn_agent_boot.trn_boot import _ntff_profile_via_ctypes

    hook = _ntff_profile_via_ctypes("/opt/axon/libaxon_pjrt.so")
    mod = types.ModuleType("antenv.axon_hooks")
    mod._hook = hook
    mod.get_axon_ntff_profile_hook = lambda: mod._hook

    def _set(h):
        mod._hook = h

    mod.set_axon_ntff_profile_hook = _set
    _sys.modules["antenv.axon_hooks"] = mod
    antenv.axon_hooks = mod


def kernel(X, alpha):
    global LAST_RESULT
    X = np.asarray(X, dtype=np.float32)
    a = float(np.asarray(alpha, dtype=np.float32).reshape(()))
    am1 = a - 1.0
    # fast path requires am1 = 2^k so all tau/am1 rescalings are exact
    assert am1 > 0 and math.log2(am1) == round(math.log2(am1)), (
        f"unsupported alpha={a}"
    )

    orig_shape = X.shape
    Xf = np.ascontiguousarray(X.reshape(-1, D))
    rows_total = Xf.shape[0]
    assert rows_total % N_CORES == 0
    rows = rows_total // N_CORES
    shards = np.split(Xf, N_CORES, axis=0)

    nc = _get_nc(am1, rows)
    in_maps = [{"X": np.ascontiguousarray(s)} for s in shards]
    if TRACE:
        _ensure_ntff_hook()
    res = None
    for attempt in range(3):
        try:
            res = run_bass_kernel_spmd(nc, in_maps, list(range(N_CORES)),
                                       trace=TRACE)
            break
        except Exception:
            # transient NRT_EXEC_UNIT_UNRECOVERABLE happens occasionally;
            # a retry recovers the device
            if attempt == 2:
                raise
            import time
            time.sleep(5.0)
    LAST_RESULT = res
    out = np.concatenate([r["OUT"] for r in res.results], axis=0)
    return np.ascontiguousarray(out.reshape(orig_shape).astype(np.float32))


# revision 27
# speedup vs baseline: 1.0195x; 1.0195x over previous
"""Entmax-bisect (alpha-entmax via 10-step bisection) on Trainium2.

Data-parallel over 8 NeuronCores: X [8, 2048, 4096] is sharded on the
leading dim (2048 rows x 4096 per core); the reduction dim stays local.
alpha is a replicated scalar folded into compile-time constants.

Math (per row, alpha=1.5 => am1=0.5):
    Xs = am1*X; mx = max(Xs); tau_lo = mx-1; tau_hi = mx-(1/d)^am1
    f(t) = sum(relu(Xs-t)^2) - 1;  10 bisection steps; out = p/sum(p)

On-device we work in the tau-hat domain (tau/am1), bit-exact w.r.t. the
reference when am1 is a power of two. Three-engine pipeline, G row-tiles
marching in lockstep through the bisection so every engine always has
independent work:
  class D pass: GPSIMD r=max(x,th)-th  ->  ACT p=(am1*r)^2 (+row-sum)
  class B pass: one fused custom DVE op p=relu((x-th)*am1)^2 (+row-sum)
  per-row scalar updates: fused custom DVE ops on [128,1] tiles
"""

import math
from operator import add as _op_add

import numpy as np

import concourse.bass as bass  # noqa: F401
import concourse.tile as tile
from concourse import bacc, mybir
from concourse.bass_utils import run_bass_kernel_spmd

N_CORES = 8
D = 4096
N_ITER = 10
P = 128

G = 5          # tiles marching together
B_MOD = 16     # class-B rotation modulus
B_THR = 1      # passes per B_MOD that run class B (fused on DVE)
NORM_ACT_MOD = 0   # >0: tiles with t % mod == 0 normalize on ACT, not DVE

TRACE = False
LAST_RESULT = None

_NC_CACHE = {}


# ---------- runtime registration of custom DVE ops ----------------------

def _register_dve_op(op_name, spec):
    from concourse import dve_ops as DO
    from concourse.dve_spec import lower, _has_src1 as has_src1
    from concourse.dve_uop import DveOpSpec

    for o in DO.OPS:
        if o.name == op_name:
            return o
    row = DO._CUSTOM_DVE_ROW_BASE + len(DO.OPS)
    assert row < 0x20
    shas = {}
    for ver in ("v3", "v4"):
        s = DveOpSpec(name=op_name, opcode=row, uops=lower(spec, ver=ver),
                      rd1_en=has_src1(spec))
        shas[ver] = s.sha(ver)
    op = DO.DveOp(op_name, spec, subdim=False, uops_sha=shas)
    DO.OPS.append(op)
    DO._SUB_OPCODE_FOR_NAME[op_name] = row
    DO.CUSTOM_DVE_SPECS[op_name] = spec
    return op


def _get_ops():
    from concourse.dve_spec import (
        Spec, Src0, Src1, C0, C1, C2, Zero, relu, select, sq,
    )

    def _ref_step(in0, in1, c0, c1, c2):
        b = np.maximum((in0.astype(np.float32) - c0) * c2, 0.0) ** 2
        b = b.astype(np.float32)
        return b, c1 + b.reshape(b.shape[0], -1).sum(axis=-1, keepdims=True)

    # out = relu((x - th)*am1)^2 ; accum = init + sum(out)
    step = _register_dve_op(
        "ENTMAX_STEP_ANT",
        Spec(body=sq(relu((Src0 - C0) * C2)), accum=_op_add, accum_init=C1,
             reference=_ref_step),
    )
    # out = select((f + imm2)*flo >= 0, tm, tlo)
    upd = _register_dve_op(
        "ENTMAX_TAU_UPD_ANT",
        Spec(body=select((Src0 + C2) * C0 >= Zero, Src1, C1),
             reference=lambda in0, in1, s0, s1, imm2: np.where(
                 (in0 + imm2) * s0 >= 0, in1, s1).astype(np.float32)),
    )
    # out = (mx - c_hi) - tlo        (per-row dm0, matching fp32 ref order)
    sub2 = _register_dve_op(
        "ENTMAX_SUB2_ANT",
        Spec(body=(Src0 - C0) - Src1,
             reference=lambda in0, in1, s0, s1, imm2: (
                 (in0.astype(np.float32) - s0) - in1).astype(np.float32)),
    )
    from concourse.dve_ops import AFFINE_THEN_ADD
    return step, upd, sub2, AFFINE_THEN_ADD


def _is_b(t_global, k):
    # class-B (fused DVE) passes; the rest are GPSIMD->ACT
    return ((k + 3 * t_global) % B_MOD) < B_THR


def _build(am1: float, rows: int):
    """Build the single-core Bass program for a [rows, D] shard."""
    f32 = mybir.dt.float32
    AF = mybir.ActivationFunctionType
    OP = mybir.AluOpType
    AX = mybir.AxisListType
    STEP, UPD, SUB2, AFF = _get_ops()

    # tau-hat domain constants (exact when am1 is a power of two)
    c_lo = 1.0 / am1
    pw = float(np.power(np.float32(1.0 / D), np.float32(am1)))
    c_hi = pw / am1

    nc = bacc.Bacc(None, target_bir_lowering=False)
    Xd = nc.declare_dram_parameter("X", [rows, D], f32, isOutput=False)
    Od = nc.declare_dram_parameter("OUT", [rows, D], f32, isOutput=True)
    ntiles = rows // P

    with tile.TileContext(nc) as tc:
        with (
            tc.tile_pool(name="xp", bufs=6) as xp,
            tc.tile_pool(name="rp", bufs=6) as rp,
            tc.tile_pool(name="st", bufs=14) as st,
        ):
            xt, rt, mx, tlo, dm0, ssum = {}, {}, {}, {}, {}, {}

            def emit_dma(t):
                xt[t] = xp.tile([P, D], f32, tag="xt", name="xt")
                nc.sync.dma_start(out=xt[t][:], in_=Xd[t * P:(t + 1) * P, :])

            def emit_max(t):
                mx[t] = st.tile([P, 1], f32, tag="mx", name="mx")
                nc.vector.reduce_max(mx[t][:], xt[t][:], axis=AX.X)

            def emit_setup(t):
                tlo[t] = st.tile([P, 1], f32, tag="tlo", name="tlo")
                nc.vector.tensor_scalar(tlo[t][:], mx[t][:], c_lo, None,
                                        OP.subtract)
                dm0[t] = st.tile([P, 1], f32, tag="dm0", name="dm0")
                nc.vector._custom_dve(SUB2, out=dm0[t][:], in0=mx[t][:],
                                      in1=tlo[t][:], s0=c_hi, s1=0.0,
                                      imm2=0.0)
                rt[t] = rp.tile([P, D], f32, tag="rt", name="rt")

            def emit_pass(t, k, force_b=False):
                # tau_m = tlo + dm0 * 2^-k   (dm halving is exact)
                tau = st.tile([P, 1], f32, tag="tm", name="tm")
                nc.vector._custom_dve(AFF, out=tau[:], in0=dm0[t][:],
                                      in1=tlo[t][:], s0=0.5 ** k, s1=0.0,
                                      imm2=0.0)
                last = k == N_ITER
                # acc: f = sum-1 for bisection steps (class B), raw sum
                # for the final pass / class A (f_bias compensates)
                acc = st.tile([P, 1], f32, tag="acc", name="acc")
                if force_b or _is_b(t, k):
                    nc.vector._custom_dve(
                        STEP, out=rt[t][:], in0=xt[t][:], s0=tau[:],
                        s1=(0.0 if last else -1.0), imm2=am1,
                        accum_out=acc[:])
                    f_bias = 0.0
                else:
                    nc.vector.tensor_scalar(rt[t][:], xt[t][:], tau[:],
                                            tau[:], OP.max, OP.subtract)
                    nc.scalar.activation(rt[t][:], rt[t][:], AF.Square,
                                         bias=0.0, scale=am1,
                                         accum_out=acc[:])
                    f_bias = -1.0
                if not last:
                    # mask = f_m >= 0  (f_lo >= 0 always; f_lo==0 rows give
                    # identical one-hot output either way, so the f_lo pass
                    # is dropped and its factor replaced by 1.0)
                    tlo_new = st.tile([P, 1], f32, tag="tlo", name="tlo")
                    nc.vector._custom_dve(
                        UPD, out=tlo_new[:], in0=acc[:], in1=tau[:],
                        s0=1.0, s1=tlo[t][:], imm2=f_bias)
                    tlo[t] = tlo_new
                else:
                    ssum[t] = acc

            def emit_teardown(t):
                rr = st.tile([P, 1], f32, tag="rr", name="rr")
                nc.vector.reciprocal(rr[:], ssum[t][:])
                if NORM_ACT_MOD > 0 and t % NORM_ACT_MOD == 0:
                    nc.scalar.mul(rt[t][:], rt[t][:], rr[:])
                else:
                    nc.vector.tensor_scalar(rt[t][:], rt[t][:], rr[:],
                                            None, OP.mult)
                nc.gpsimd.dma_start(out=Od[t * P:(t + 1) * P, :],
                                    in_=rt[t][:])

            # Skewed software pipeline: tile t runs pass k at step
            # s = off[t] + k, so ~N_ITER/stride tiles are mid-flight at any
            # step and setup/teardown work is spread evenly. Stride 1 at
            # both ends compresses the pipeline ramp/drain.
            offs = [2 * t for t in range(ntiles)]
            for s in range(-1, (offs[-1] if offs else 0) + N_ITER + 2):
                for t in range(ntiles):
                    k = s - offs[t]
                    if k == -1:
                        emit_dma(t)
                    elif k == 0:
                        emit_max(t)
                        emit_setup(t)
                    elif 1 <= k <= N_ITER:
                        # the drain edge runs solo: the fused DVE pass has a
                        # shorter serial chain than relu->ACT there
                        solo = t == ntiles - 1 and k >= 9
                        emit_pass(t, k, force_b=solo)
                    elif k == N_ITER + 1:
                        emit_teardown(t)

    nc.finalize()
    return nc


def _get_nc(am1: float, rows: int):
    key = (am1, rows, G, B_MOD, B_THR, NORM_ACT_MOD)
    if key not in _NC_CACHE:
        _NC_CACHE[key] = _build(am1, rows)
    return _NC_CACHE[key]


def _ensure_ntff_hook():
    """Register the NTFF profile hook that bass_utils needs for trace=True
    under axon (this image's antenv lacks axon_hooks; build it from the
    boot shim's ctypes driver). Also neuter the S3 artifact upload."""
    import sys as _sys
    import types

    import antenv
    import concourse.bass_utils as _bu

    _bu.upload_artifacts = lambda tmpdir: str(tmpdir)
    try:
        from antenv import axon_hooks  # noqa: F401
        return
    except ImportError:
        pass
    from trn_agent_boot.trn_boot import _ntff_profile_via_ctypes

    hook = _ntff_profile_via_ctypes("/opt/axon/libaxon_pjrt.so")
    mod = types.ModuleType("antenv.axon_hooks")
    mod._hook = hook
    mod.get_axon_ntff_profile_hook = lambda: mod._hook

    def _set(h):
        mod._hook = h

    mod.set_axon_ntff_profile_hook = _set
    _sys.modules["antenv.axon_hooks"] = mod
    antenv.axon_hooks = mod


def kernel(X, alpha):
    global LAST_RESULT
    X = np.asarray(X, dtype=np.float32)
    a = float(np.asarray(alpha, dtype=np.float32).reshape(()))
    am1 = a - 1.0
    # fast path requires am1 = 2^k so all tau/am1 rescalings are exact
    assert am1 > 0 and math.log2(am1) == round(math.log2(am1)), (
        f"unsupported alpha={a}"
    )

    orig_shape = X.shape
    Xf = np.ascontiguousarray(X.reshape(-1, D))
    rows_total = Xf.shape[0]
    assert rows_total % N_CORES == 0
    rows = rows_total // N_CORES
    shards = np.split(Xf, N_CORES, axis=0)

    nc = _get_nc(am1, rows)
    in_maps = [{"X": np.ascontiguousarray(s)} for s in shards]
    if TRACE:
        _ensure_ntff_hook()
    res = None
    for attempt in range(3):
        try:
            res = run_bass_kernel_spmd(nc, in_maps, list(range(N_CORES)),
                                       trace=TRACE)
            break
        except Exception:
            # transient NRT_EXEC_UNIT_UNRECOVERABLE happens occasionally;
            # a retry recovers the device
            if attempt == 2:
                raise
            import time
            time.sleep(5.0)
    LAST_RESULT = res
    out = np.concatenate([r["OUT"] for r in res.results], axis=0)
    return np.ascontiguousarray(out.reshape(orig_shape).astype(np.float32))


# revision 32
# speedup vs baseline: 1.0463x; 1.0263x over previous
"""Entmax-bisect (alpha-entmax via 10-step bisection) on Trainium2.

Data-parallel over 8 NeuronCores: X [8, 2048, 4096] is sharded on the
leading dim (2048 rows x 4096 per core); the reduction dim stays local.
alpha is a replicated scalar folded into compile-time constants.

Math (per row, alpha=1.5 => am1=0.5):
    Xs = am1*X; mx = max(Xs); tau_lo = mx-1; tau_hi = mx-(1/d)^am1
    f(t) = sum(relu(Xs-t)^2) - 1;  10 bisection steps; out = p/sum(p)

On-device we work in the tau-hat domain (tau/am1), bit-exact w.r.t. the
reference when am1 is a power of two. Three-engine pipeline, G row-tiles
marching in lockstep through the bisection so every engine always has
independent work:
  class D pass: GPSIMD r=max(x,th)-th  ->  ACT p=(am1*r)^2 (+row-sum)
  class B pass: one fused custom DVE op p=relu((x-th)*am1)^2 (+row-sum)
  per-row scalar updates: fused custom DVE ops on [128,1] tiles
"""

import math
from operator import add as _op_add

import numpy as np

import concourse.bass as bass  # noqa: F401
import concourse.tile as tile
from concourse import bacc, mybir
from concourse.bass_utils import run_bass_kernel_spmd

N_CORES = 8
D = 4096
N_ITER = 10
P = 128

G = 5          # tiles marching together
B_MOD = 16     # class-B rotation modulus
B_THR = 1      # passes per B_MOD that run class B (fused on DVE)
NORM_ACT_MOD = 0   # >0: tiles with t % mod == 0 normalize on ACT, not DVE

TRACE = False
LAST_RESULT = None

_NC_CACHE = {}


# ---------- runtime registration of custom DVE ops ----------------------

def _register_dve_op(op_name, spec):
    from concourse import dve_ops as DO
    from concourse.dve_spec import lower, _has_src1 as has_src1
    from concourse.dve_uop import DveOpSpec

    for o in DO.OPS:
        if o.name == op_name:
            return o
    row = DO._CUSTOM_DVE_ROW_BASE + len(DO.OPS)
    assert row < 0x20
    shas = {}
    for ver in ("v3", "v4"):
        s = DveOpSpec(name=op_name, opcode=row, uops=lower(spec, ver=ver),
                      rd1_en=has_src1(spec))
        shas[ver] = s.sha(ver)
    op = DO.DveOp(op_name, spec, subdim=False, uops_sha=shas)
    DO.OPS.append(op)
    DO._SUB_OPCODE_FOR_NAME[op_name] = row
    DO.CUSTOM_DVE_SPECS[op_name] = spec
    return op


def _get_ops():
    from concourse.dve_spec import (
        Spec, Src0, Src1, C0, C1, C2, Zero, relu, select, sq,
    )

    def _ref_step(in0, in1, c0, c1, c2):
        b = np.maximum((in0.astype(np.float32) - c0) * c2, 0.0) ** 2
        b = b.astype(np.float32)
        return b, c1 + b.reshape(b.shape[0], -1).sum(axis=-1, keepdims=True)

    # out = relu((x - th)*am1)^2 ; accum = init + sum(out)
    step = _register_dve_op(
        "ENTMAX_STEP_ANT",
        Spec(body=sq(relu((Src0 - C0) * C2)), accum=_op_add, accum_init=C1,
             reference=_ref_step),
    )
    # out = select((f + imm2)*flo >= 0, tm, tlo)
    upd = _register_dve_op(
        "ENTMAX_TAU_UPD_ANT",
        Spec(body=select((Src0 + C2) * C0 >= Zero, Src1, C1),
             reference=lambda in0, in1, s0, s1, imm2: np.where(
                 (in0 + imm2) * s0 >= 0, in1, s1).astype(np.float32)),
    )
    # out = select((accA + accB + imm2) >= 0, tm, tlo)   (split-pass update)
    upd2 = _register_dve_op(
        "ENTMAX_TAU_UPD2_ANT",
        Spec(body=select((Src0 + C0) + C2 >= Zero, Src1, C1),
             reference=lambda in0, in1, s0, s1, imm2: np.where(
                 (in0 + s0) + imm2 >= 0, in1, s1).astype(np.float32)),
    )
    # out = (mx - c_hi) - tlo        (per-row dm0, matching fp32 ref order)
    sub2 = _register_dve_op(
        "ENTMAX_SUB2_ANT",
        Spec(body=(Src0 - C0) - Src1,
             reference=lambda in0, in1, s0, s1, imm2: (
                 (in0.astype(np.float32) - s0) - in1).astype(np.float32)),
    )
    from concourse.dve_ops import AFFINE_THEN_ADD
    return step, upd, upd2, sub2, AFFINE_THEN_ADD


def _is_b(t_global, k):
    # class-B (fused DVE) passes; the rest are GPSIMD->ACT
    return ((k + 3 * t_global) % B_MOD) < B_THR


def _build(am1: float, rows: int):
    """Build the single-core Bass program for a [rows, D] shard."""
    f32 = mybir.dt.float32
    AF = mybir.ActivationFunctionType
    OP = mybir.AluOpType
    AX = mybir.AxisListType
    STEP, UPD, UPD2, SUB2, AFF = _get_ops()

    # tau-hat domain constants (exact when am1 is a power of two)
    c_lo = 1.0 / am1
    pw = float(np.power(np.float32(1.0 / D), np.float32(am1)))
    c_hi = pw / am1

    nc = bacc.Bacc(None, target_bir_lowering=False)
    Xd = nc.declare_dram_parameter("X", [rows, D], f32, isOutput=False)
    Od = nc.declare_dram_parameter("OUT", [rows, D], f32, isOutput=True)
    ntiles = rows // P
    HC = D // 2

    def _is_half(t, k):
        # exactly one mid-flight tile per pipeline step runs a split pass
        return (t + k) % 5 == 0

    with tile.TileContext(nc) as tc:
        with (
            tc.tile_pool(name="xp", bufs=6) as xp,
            tc.tile_pool(name="rp", bufs=6) as rp,
            tc.tile_pool(name="st", bufs=14) as st,
        ):
            xt, rt, mx, tlo, dm0, ssum = {}, {}, {}, {}, {}, {}

            def emit_dma(t):
                xt[t] = xp.tile([P, D], f32, tag="xt", name="xt")
                nc.sync.dma_start(out=xt[t][:], in_=Xd[t * P:(t + 1) * P, :])

            def emit_max(t):
                mx[t] = st.tile([P, 1], f32, tag="mx", name="mx")
                nc.vector.reduce_max(mx[t][:], xt[t][:], axis=AX.X)

            def emit_setup(t):
                tlo[t] = st.tile([P, 1], f32, tag="tlo", name="tlo")
                nc.vector.tensor_scalar(tlo[t][:], mx[t][:], c_lo, None,
                                        OP.subtract)
                dm0[t] = st.tile([P, 1], f32, tag="dm0", name="dm0")
                nc.vector._custom_dve(SUB2, out=dm0[t][:], in0=mx[t][:],
                                      in1=tlo[t][:], s0=c_hi, s1=0.0,
                                      imm2=0.0)
                rt[t] = rp.tile([P, D], f32, tag="rt", name="rt")

            def emit_pass(t, k, force_b=False):
                # tau_m = tlo + dm0 * 2^-k   (dm halving is exact)
                tau = st.tile([P, 1], f32, tag="tm", name="tm")
                nc.vector._custom_dve(AFF, out=tau[:], in0=dm0[t][:],
                                      in1=tlo[t][:], s0=0.5 ** k, s1=0.0,
                                      imm2=0.0)
                last = k == N_ITER
                # mask = f_m >= 0  (f_lo >= 0 always; f_lo==0 rows give
                # identical one-hot output either way, so the f_lo pass is
                # dropped and its factor replaced by 1.0)
                if force_b:
                    # fused single-engine pass (shortest serial chain)
                    acc = st.tile([P, 1], f32, tag="acc", name="acc")
                    nc.vector._custom_dve(
                        STEP, out=rt[t][:], in0=xt[t][:], s0=tau[:],
                        s1=(0.0 if last else -1.0), imm2=am1,
                        accum_out=acc[:])
                    if not last:
                        tlo_new = st.tile([P, 1], f32, tag="tlo", name="tlo")
                        nc.vector._custom_dve(
                            UPD, out=tlo_new[:], in0=acc[:], in1=tau[:],
                            s0=1.0, s1=tlo[t][:], imm2=0.0)
                        tlo[t] = tlo_new
                    else:
                        ssum[t] = acc
                elif _is_half(t, k):
                    # split pass: low half fused on DVE, high half via
                    # relu(DVE 2x) -> Square+accum(ACT); balances per-step
                    # engine loads (kills the whole-pass alternation loss)
                    accA = st.tile([P, 1], f32, tag="acc", name="accA")
                    accB = st.tile([P, 1], f32, tag="accB", name="accB")
                    nc.vector.tensor_scalar(rt[t][:, HC:], xt[t][:, HC:],
                                            tau[:], tau[:], OP.max,
                                            OP.subtract)
                    nc.scalar.activation(rt[t][:, HC:], rt[t][:, HC:],
                                         AF.Square, bias=0.0, scale=am1,
                                         accum_out=accA[:])
                    nc.vector._custom_dve(
                        STEP, out=rt[t][:, :HC], in0=xt[t][:, :HC],
                        s0=tau[:], s1=0.0, imm2=am1, accum_out=accB[:])
                    if not last:
                        tlo_new = st.tile([P, 1], f32, tag="tlo", name="tlo")
                        nc.vector._custom_dve(
                            UPD2, out=tlo_new[:], in0=accA[:], in1=tau[:],
                            s0=accB[:], s1=tlo[t][:], imm2=-1.0)
                        tlo[t] = tlo_new
                    else:
                        ss = st.tile([P, 1], f32, tag="acc", name="ss")
                        nc.vector.tensor_add(ss[:], accA[:], accB[:])
                        ssum[t] = ss
                else:
                    acc = st.tile([P, 1], f32, tag="acc", name="acc")
                    nc.vector.tensor_scalar(rt[t][:], xt[t][:], tau[:],
                                            tau[:], OP.max, OP.subtract)
                    nc.scalar.activation(rt[t][:], rt[t][:], AF.Square,
                                         bias=0.0, scale=am1,
                                         accum_out=acc[:])
                    if not last:
                        tlo_new = st.tile([P, 1], f32, tag="tlo", name="tlo")
                        nc.vector._custom_dve(
                            UPD, out=tlo_new[:], in0=acc[:], in1=tau[:],
                            s0=1.0, s1=tlo[t][:], imm2=-1.0)
                        tlo[t] = tlo_new
                    else:
                        ssum[t] = acc

            def emit_teardown(t):
                rr = st.tile([P, 1], f32, tag="rr", name="rr")
                nc.vector.reciprocal(rr[:], ssum[t][:])
                if NORM_ACT_MOD > 0 and t % NORM_ACT_MOD == 0:
                    nc.scalar.mul(rt[t][:], rt[t][:], rr[:])
                else:
                    nc.vector.tensor_scalar(rt[t][:], rt[t][:], rr[:],
                                            None, OP.mult)
                nc.gpsimd.dma_start(out=Od[t * P:(t + 1) * P, :],
                                    in_=rt[t][:])

            # Skewed software pipeline: tile t runs pass k at step
            # s = off[t] + k, so ~N_ITER/stride tiles are mid-flight at any
            # step and setup/teardown work is spread evenly. Stride 1 at
            # both ends compresses the pipeline ramp/drain.
            offs = [2 * t for t in range(ntiles)]
            for s in range(-1, (offs[-1] if offs else 0) + N_ITER + 2):
                for t in range(ntiles):
                    k = s - offs[t]
                    if k == -1:
                        emit_dma(t)
                    elif k == 0:
                        emit_max(t)
                        emit_setup(t)
                    elif 1 <= k <= N_ITER:
                        # the drain edge runs solo: the fused DVE pass has a
                        # shorter serial chain than relu->ACT there
                        solo = t == ntiles - 1 and k >= 9
                        emit_pass(t, k, force_b=solo)
                    elif k == N_ITER + 1:
                        emit_teardown(t)

    nc.finalize()
    return nc


def _get_nc(am1: float, rows: int):
    key = (am1, rows, G, B_MOD, B_THR, NORM_ACT_MOD)
    if key not in _NC_CACHE:
        _NC_CACHE[key] = _build(am1, rows)
    return _NC_CACHE[key]


def _ensure_ntff_hook():
    """Register the NTFF profile hook that bass_utils needs for trace=True
    under axon (this image's antenv lacks axon_hooks; build it from the
    boot shim's ctypes driver). Also neuter the S3 artifact upload."""
    import sys as _sys
    import types

    import antenv
    import concourse.bass_utils as _bu

    _bu.upload_artifacts = lambda tmpdir: str(tmpdir)
    try:
        from antenv import axon_hooks  # noqa: F401
        return
    except ImportError:
        pass
    from trn_agent_boot.trn_boot import _ntff_profile_via_ctypes

    hook = _ntff_profile_via_ctypes("/opt/axon/libaxon_pjrt.so")
    mod = types.ModuleType("antenv.axon_hooks")
    mod._hook = hook
    mod.get_axon_ntff_profile_hook = lambda: mod._hook

    def _set(h):
        mod._hook = h

    mod.set_axon_ntff_profile_hook = _set
    _sys.modules["antenv.axon_hooks"] = mod
    antenv.axon_hooks = mod


def kernel(X, alpha):
    global LAST_RESULT
    X = np.asarray(X, dtype=np.float32)
    a = float(np.asarray(alpha, dtype=np.float32).reshape(()))
    am1 = a - 1.0
    # fast path requires am1 = 2^k so all tau/am1 rescalings are exact
    assert am1 > 0 and math.log2(am1) == round(math.log2(am1)), (
        f"unsupported alpha={a}"
    )

    orig_shape = X.shape
    Xf = np.ascontiguousarray(X.reshape(-1, D))
    rows_total = Xf.shape[0]
    assert rows_total % N_CORES == 0
    rows = rows_total // N_CORES
    shards = np.split(Xf, N_CORES, axis=0)

    nc = _get_nc(am1, rows)
    in_maps = [{"X": np.ascontiguousarray(s)} for s in shards]
    if TRACE:
        _ensure_ntff_hook()
    res = None
    for attempt in range(3):
        try:
            res = run_bass_kernel_spmd(nc, in_maps, list(range(N_CORES)),
                                       trace=TRACE)
            break
        except Exception:
            # transient NRT_EXEC_UNIT_UNRECOVERABLE happens occasionally;
            # a retry recovers the device
            if attempt == 2:
                raise
            import time
            time.sleep(5.0)
    LAST_RESULT = res
    out = np.concatenate([r["OUT"] for r in res.results], axis=0)
    return np.ascontiguousarray(out.reshape(orig_shape).astype(np.float32))


# revision 33
# speedup vs baseline: 1.0550x; 1.0083x over previous
"""Entmax-bisect (alpha-entmax via 10-step bisection) on Trainium2.

Data-parallel over 8 NeuronCores: X [8, 2048, 4096] is sharded on the
leading dim (2048 rows x 4096 per core); the reduction dim stays local.
alpha is a replicated scalar folded into compile-time constants.

Math (per row, alpha=1.5 => am1=0.5):
    Xs = am1*X; mx = max(Xs); tau_lo = mx-1; tau_hi = mx-(1/d)^am1
    f(t) = sum(relu(Xs-t)^2) - 1;  10 bisection steps; out = p/sum(p)

On-device we work in the tau-hat domain (tau/am1), bit-exact w.r.t. the
reference when am1 is a power of two. Three-engine pipeline, G row-tiles
marching in lockstep through the bisection so every engine always has
independent work:
  class D pass: GPSIMD r=max(x,th)-th  ->  ACT p=(am1*r)^2 (+row-sum)
  class B pass: one fused custom DVE op p=relu((x-th)*am1)^2 (+row-sum)
  per-row scalar updates: fused custom DVE ops on [128,1] tiles
"""

import math
from operator import add as _op_add

import numpy as np

import concourse.bass as bass  # noqa: F401
import concourse.tile as tile
from concourse import bacc, mybir
from concourse.bass_utils import run_bass_kernel_spmd

N_CORES = 8
D = 4096
N_ITER = 10
P = 128

G = 5          # tiles marching together
B_MOD = 16     # class-B rotation modulus
B_THR = 1      # passes per B_MOD that run class B (fused on DVE)
NORM_ACT_MOD = 0   # >0: tiles with t % mod == 0 normalize on ACT, not DVE

TRACE = False
LAST_RESULT = None

_NC_CACHE = {}


# ---------- runtime registration of custom DVE ops ----------------------

def _register_dve_op(op_name, spec):
    from concourse import dve_ops as DO
    from concourse.dve_spec import lower, _has_src1 as has_src1
    from concourse.dve_uop import DveOpSpec

    for o in DO.OPS:
        if o.name == op_name:
            return o
    row = DO._CUSTOM_DVE_ROW_BASE + len(DO.OPS)
    assert row < 0x20
    shas = {}
    for ver in ("v3", "v4"):
        s = DveOpSpec(name=op_name, opcode=row, uops=lower(spec, ver=ver),
                      rd1_en=has_src1(spec))
        shas[ver] = s.sha(ver)
    op = DO.DveOp(op_name, spec, subdim=False, uops_sha=shas)
    DO.OPS.append(op)
    DO._SUB_OPCODE_FOR_NAME[op_name] = row
    DO.CUSTOM_DVE_SPECS[op_name] = spec
    return op


def _get_ops():
    from concourse.dve_spec import (
        Spec, Src0, Src1, C0, C1, C2, Zero, relu, select, sq,
    )

    def _ref_step(in0, in1, c0, c1, c2):
        b = np.maximum((in0.astype(np.float32) - c0) * c2, 0.0) ** 2
        b = b.astype(np.float32)
        return b, c1 + b.reshape(b.shape[0], -1).sum(axis=-1, keepdims=True)

    # out = relu((x - th)*am1)^2 ; accum = init + sum(out)
    step = _register_dve_op(
        "ENTMAX_STEP_ANT",
        Spec(body=sq(relu((Src0 - C0) * C2)), accum=_op_add, accum_init=C1,
             reference=_ref_step),
    )
    # out = select((f + imm2)*flo >= 0, tm, tlo)
    upd = _register_dve_op(
        "ENTMAX_TAU_UPD_ANT",
        Spec(body=select((Src0 + C2) * C0 >= Zero, Src1, C1),
             reference=lambda in0, in1, s0, s1, imm2: np.where(
                 (in0 + imm2) * s0 >= 0, in1, s1).astype(np.float32)),
    )
    # out = select((accA + accB + imm2) >= 0, tm, tlo)   (split-pass update)
    upd2 = _register_dve_op(
        "ENTMAX_TAU_UPD2_ANT",
        Spec(body=select((Src0 + C0) + C2 >= Zero, Src1, C1),
             reference=lambda in0, in1, s0, s1, imm2: np.where(
                 (in0 + s0) + imm2 >= 0, in1, s1).astype(np.float32)),
    )
    # out = (mx - c_hi) - tlo        (per-row dm0, matching fp32 ref order)
    sub2 = _register_dve_op(
        "ENTMAX_SUB2_ANT",
        Spec(body=(Src0 - C0) - Src1,
             reference=lambda in0, in1, s0, s1, imm2: (
                 (in0.astype(np.float32) - s0) - in1).astype(np.float32)),
    )
    from concourse.dve_ops import AFFINE_THEN_ADD
    return step, upd, upd2, sub2, AFFINE_THEN_ADD


def _is_b(t_global, k):
    # class-B (fused DVE) passes; the rest are GPSIMD->ACT
    return ((k + 3 * t_global) % B_MOD) < B_THR


def _build(am1: float, rows: int):
    """Build the single-core Bass program for a [rows, D] shard."""
    f32 = mybir.dt.float32
    AF = mybir.ActivationFunctionType
    OP = mybir.AluOpType
    AX = mybir.AxisListType
    STEP, UPD, UPD2, SUB2, AFF = _get_ops()

    # tau-hat domain constants (exact when am1 is a power of two)
    c_lo = 1.0 / am1
    pw = float(np.power(np.float32(1.0 / D), np.float32(am1)))
    c_hi = pw / am1

    nc = bacc.Bacc(None, target_bir_lowering=False)
    Xd = nc.declare_dram_parameter("X", [rows, D], f32, isOutput=False)
    Od = nc.declare_dram_parameter("OUT", [rows, D], f32, isOutput=True)
    ntiles = rows // P
    HC = 1280  # split-pass boundary: [0:HC) fused on DVE, [HC:D) on ACT

    def _is_half(t, k):
        # exactly one mid-flight tile per pipeline step runs a split pass
        return (t + k) % 5 == 0

    with tile.TileContext(nc) as tc:
        with (
            tc.tile_pool(name="xp", bufs=6) as xp,
            tc.tile_pool(name="rp", bufs=6) as rp,
            tc.tile_pool(name="st", bufs=14) as st,
        ):
            xt, rt, mx, tlo, dm0, ssum = {}, {}, {}, {}, {}, {}

            def emit_dma(t):
                xt[t] = xp.tile([P, D], f32, tag="xt", name="xt")
                nc.sync.dma_start(out=xt[t][:], in_=Xd[t * P:(t + 1) * P, :])

            def emit_max(t):
                mx[t] = st.tile([P, 1], f32, tag="mx", name="mx")
                nc.vector.reduce_max(mx[t][:], xt[t][:], axis=AX.X)

            def emit_setup(t):
                tlo[t] = st.tile([P, 1], f32, tag="tlo", name="tlo")
                nc.vector.tensor_scalar(tlo[t][:], mx[t][:], c_lo, None,
                                        OP.subtract)
                dm0[t] = st.tile([P, 1], f32, tag="dm0", name="dm0")
                nc.vector._custom_dve(SUB2, out=dm0[t][:], in0=mx[t][:],
                                      in1=tlo[t][:], s0=c_hi, s1=0.0,
                                      imm2=0.0)
                rt[t] = rp.tile([P, D], f32, tag="rt", name="rt")

            def emit_pass(t, k, force_b=False):
                # tau_m = tlo + dm0 * 2^-k   (dm halving is exact)
                tau = st.tile([P, 1], f32, tag="tm", name="tm")
                nc.vector._custom_dve(AFF, out=tau[:], in0=dm0[t][:],
                                      in1=tlo[t][:], s0=0.5 ** k, s1=0.0,
                                      imm2=0.0)
                last = k == N_ITER
                # mask = f_m >= 0  (f_lo >= 0 always; f_lo==0 rows give
                # identical one-hot output either way, so the f_lo pass is
                # dropped and its factor replaced by 1.0)
                if force_b:
                    # fused single-engine pass (shortest serial chain)
                    acc = st.tile([P, 1], f32, tag="acc", name="acc")
                    nc.vector._custom_dve(
                        STEP, out=rt[t][:], in0=xt[t][:], s0=tau[:],
                        s1=(0.0 if last else -1.0), imm2=am1,
                        accum_out=acc[:])
                    if not last:
                        tlo_new = st.tile([P, 1], f32, tag="tlo", name="tlo")
                        nc.vector._custom_dve(
                            UPD, out=tlo_new[:], in0=acc[:], in1=tau[:],
                            s0=1.0, s1=tlo[t][:], imm2=0.0)
                        tlo[t] = tlo_new
                    else:
                        ssum[t] = acc
                elif _is_half(t, k):
                    # split pass: low half fused on DVE, high half via
                    # relu(DVE 2x) -> Square+accum(ACT); balances per-step
                    # engine loads (kills the whole-pass alternation loss)
                    accA = st.tile([P, 1], f32, tag="acc", name="accA")
                    accB = st.tile([P, 1], f32, tag="accB", name="accB")
                    nc.vector.tensor_scalar(rt[t][:, HC:], xt[t][:, HC:],
                                            tau[:], tau[:], OP.max,
                                            OP.subtract)
                    nc.scalar.activation(rt[t][:, HC:], rt[t][:, HC:],
                                         AF.Square, bias=0.0, scale=am1,
                                         accum_out=accA[:])
                    nc.vector._custom_dve(
                        STEP, out=rt[t][:, :HC], in0=xt[t][:, :HC],
                        s0=tau[:], s1=0.0, imm2=am1, accum_out=accB[:])
                    if not last:
                        tlo_new = st.tile([P, 1], f32, tag="tlo", name="tlo")
                        nc.vector._custom_dve(
                            UPD2, out=tlo_new[:], in0=accA[:], in1=tau[:],
                            s0=accB[:], s1=tlo[t][:], imm2=-1.0)
                        tlo[t] = tlo_new
                    else:
                        ss = st.tile([P, 1], f32, tag="acc", name="ss")
                        nc.vector.tensor_add(ss[:], accA[:], accB[:])
                        ssum[t] = ss
                else:
                    acc = st.tile([P, 1], f32, tag="acc", name="acc")
                    nc.vector.tensor_scalar(rt[t][:], xt[t][:], tau[:],
                                            tau[:], OP.max, OP.subtract)
                    nc.scalar.activation(rt[t][:], rt[t][:], AF.Square,
                                         bias=0.0, scale=am1,
                                         accum_out=acc[:])
                    if not last:
                        tlo_new = st.tile([P, 1], f32, tag="tlo", name="tlo")
                        nc.vector._custom_dve(
                            UPD, out=tlo_new[:], in0=acc[:], in1=tau[:],
                            s0=1.0, s1=tlo[t][:], imm2=-1.0)
                        tlo[t] = tlo_new
                    else:
                        ssum[t] = acc

            def emit_teardown(t):
                rr = st.tile([P, 1], f32, tag="rr", name="rr")
                nc.vector.reciprocal(rr[:], ssum[t][:])
                if NORM_ACT_MOD > 0 and t % NORM_ACT_MOD == 0:
                    nc.scalar.mul(rt[t][:], rt[t][:], rr[:])
                else:
                    nc.vector.tensor_scalar(rt[t][:], rt[t][:], rr[:],
                                            None, OP.mult)
                nc.gpsimd.dma_start(out=Od[t * P:(t + 1) * P, :],
                                    in_=rt[t][:])

            # Skewed software pipeline: tile t runs pass k at step
            # s = off[t] + k, so ~N_ITER/stride tiles are mid-flight at any
            # step and setup/teardown work is spread evenly. Stride 1 at
            # both ends compresses the pipeline ramp/drain.
            offs = [2 * t for t in range(ntiles)]
            for s in range(-1, (offs[-1] if offs else 0) + N_ITER + 2):
                for t in range(ntiles):
                    k = s - offs[t]
                    if k == -1:
                        emit_dma(t)
                    elif k == 0:
                        emit_max(t)
                        emit_setup(t)
                    elif 1 <= k <= N_ITER:
                        # the drain edge runs solo: the fused DVE pass has a
                        # shorter serial chain than relu->ACT there
                        solo = t == ntiles - 1 and k >= 9
                        emit_pass(t, k, force_b=solo)
                    elif k == N_ITER + 1:
                        emit_teardown(t)

    nc.finalize()
    return nc


def _get_nc(am1: float, rows: int):
    key = (am1, rows, G, B_MOD, B_THR, NORM_ACT_MOD)
    if key not in _NC_CACHE:
        _NC_CACHE[key] = _build(am1, rows)
    return _NC_CACHE[key]


def _ensure_ntff_hook():
    """Register the NTFF profile hook that bass_utils needs for trace=True
    under axon (this image's antenv lacks axon_hooks; build it from the
    boot shim's ctypes driver). Also neuter the S3 artifact upload."""
    import sys as _sys
    import types

    import antenv
    import concourse.bass_utils as _bu

    _bu.upload_artifacts = lambda tmpdir: str(tmpdir)
    try:
        from antenv import axon_hooks  # noqa: F401
        return
    except ImportError:
        pass
    from trn_agent_boot.trn_boot import _ntff_profile_via_ctypes

    hook = _ntff_profile_via_ctypes("/opt/axon/libaxon_pjrt.so")
    mod = types.ModuleType("antenv.axon_hooks")
    mod._hook = hook
    mod.get_axon_ntff_profile_hook = lambda: mod._hook

    def _set(h):
        mod._hook = h

    mod.set_axon_ntff_profile_hook = _set
    _sys.modules["antenv.axon_hooks"] = mod
    antenv.axon_hooks = mod


def kernel(X, alpha):
    global LAST_RESULT
    X = np.asarray(X, dtype=np.float32)
    a = float(np.asarray(alpha, dtype=np.float32).reshape(()))
    am1 = a - 1.0
    # fast path requires am1 = 2^k so all tau/am1 rescalings are exact
    assert am1 > 0 and math.log2(am1) == round(math.log2(am1)), (
        f"unsupported alpha={a}"
    )

    orig_shape = X.shape
    Xf = np.ascontiguousarray(X.reshape(-1, D))
    rows_total = Xf.shape[0]
    assert rows_total % N_CORES == 0
    rows = rows_total // N_CORES
    shards = np.split(Xf, N_CORES, axis=0)

    nc = _get_nc(am1, rows)
    in_maps = [{"X": np.ascontiguousarray(s)} for s in shards]
    if TRACE:
        _ensure_ntff_hook()
    res = None
    for attempt in range(3):
        try:
            res = run_bass_kernel_spmd(nc, in_maps, list(range(N_CORES)),
                                       trace=TRACE)
            break
        except Exception:
            # transient NRT_EXEC_UNIT_UNRECOVERABLE happens occasionally;
            # a retry recovers the device
            if attempt == 2:
                raise
            import time
            time.sleep(5.0)
    LAST_RESULT = res
    out = np.concatenate([r["OUT"] for r in res.results], axis=0)
    return np.ascontiguousarray(out.reshape(orig_shape).astype(np.float32))
